# revision 15
# baseline (speedup 1.0000x reference)
"""Trainium2 Bass kernel for nn_DABConv (deformable attention-ish conv).

Data-parallel over batch: 8 samples -> 8 NeuronCores, one sample per core.

Per-core pipeline (sample = x[C=128, H=64, W=64] fp32):
  A. load x, build zero-padded bf16 image xp [128, 68*68] (pad=2)
  B. build channel-last "row-pair" image x2cl in DRAM:
     x2cl[(r,c)] = [x_cl[r,c](128ch), x_cl[r+1,c](128ch)] bf16, so one
     1KB gathered element = a full 2x2 bilinear patch.
  C. convs (std 128ch, offset+modulator 31ch) as 9 accumulating shifted
     matmuls each, bf16.  Offset/mod conv is emitted FIRST (it gates the
     gather loop); the std conv is emitted after phase E so the PE chews
     it while gathers run.
  D. PE-transpose offset/mod output to position-major layout pm.
  E. index + bilinear-weight math on DVE (position-major, fp32),
     fold mask*corner-gate into the 4 corner scales; corner scales are
     written corner-interleaved and pair-duplicated (sall) so the
     phase-G combine multiplies run with packed last-dim APs (DVE 2x).
  G. per (wave, tap): ONE batched indirect DMA gathers 16 blocks x 128
     patches (2048 x 1KB) -> 3 DVE ops (scaled corner mult + 2 tree
     adds) -> PE transpose to channel-major -> accumulating def-conv
     matmul into PSUM.
  H. fused 1x1 conv over [x_std ; x_def], biases folded host-side.
"""

import numpy as np
import ml_dtypes
from contextlib import ExitStack

import concourse.bass as bass
import concourse.bacc as bacc
import concourse.mybir as mybir
from concourse.tile import TileContext
from concourse.bass_utils import run_bass_kernel_spmd

AF = mybir.ActivationFunctionType
OP = mybir.AluOpType
F32 = mybir.dt.float32
BF16 = mybir.dt.bfloat16
I16 = mybir.dt.int16
NPBF = ml_dtypes.bfloat16

P = 128
H = 64
HP = 68          # padded image side (pad=2 each side)
NP = H * H       # 4096 output positions
NPAD = HP * HP   # 4624 padded positions
NBLK = 37        # ceil(4624/128)
K = 9
NW = 2           # waves over position blocks (PSUM capacity)
WBLK = 16        # 128-position blocks per wave
MAGIC = 12582912.0  # 2**23 + 2**22: float32 round-to-int trick
N_CORES = 8


def _r3(ap, inner):
    """[p, (a b)] -> [p, a, b] with b=inner."""
    return ap.rearrange("p (a b) -> p a b", b=inner)


def build_nc():
    nc = bacc.Bacc("TRN2", target_bir_lowering=False, debug=False)

    x_d = nc.dram_tensor("x", [P, NP], F32, kind="ExternalInput")
    wstd_d = nc.dram_tensor("wstd", [K, P, P], BF16, kind="ExternalInput")
    wom_d = nc.dram_tensor("wom", [K, P, 32], BF16, kind="ExternalInput")
    wdef_d = nc.dram_tensor("wdef", [K, P, P], BF16, kind="ExternalInput")
    wfus_d = nc.dram_tensor("wfus", [2, P, P], BF16, kind="ExternalInput")
    bfus_d = nc.dram_tensor("bfus", [P, 1], F32, kind="ExternalInput")
    bom_d = nc.dram_tensor("bom", [32, 1], F32, kind="ExternalInput")
    yb_d = nc.dram_tensor("yb", [P, K * 32], F32, kind="ExternalInput")
    xb_d = nc.dram_tensor("xb", [P, K * 32], F32, kind="ExternalInput")
    idn_d = nc.dram_tensor("idn", [P, P], BF16, kind="ExternalInput")
    idnf_d = nc.dram_tensor("idnf", [P, P], F32, kind="ExternalInput")
    out_d = nc.dram_tensor("out", [P, NP], F32, kind="ExternalOutput")
    # internal scratch: channel-last 2x2-patch image; row j = padded pos
    # (r,c) holds [x(r,c) | x(r+1,c) | x(r,c+1) | x(r+1,c+1)] x 128ch bf16.
    x4_d = nc.dram_tensor("x4cl", [NBLK * 128, 512], BF16)
    # int16 gather indices, position-major (DRAM bounce for the rewrap)
    idxd_d = nc.dram_tensor("idxd", [P, K * 32], I16)

    with TileContext(nc) as tc, ExitStack() as top:
        const = top.enter_context(tc.tile_pool(name="const", bufs=1))
        main = top.enter_context(tc.tile_pool(name="main", bufs=1))
        ps_tr = top.enter_context(tc.tile_pool(name="ps_tr", bufs=2, space="PSUM"))

        # ---- const loads ----
        wstd = const.tile([P, K * P], BF16, tag="wstd", name="wstd")
        nc.sync.dma_start(_r3(wstd, P), wstd_d[:, :, :].transpose([1, 0, 2]))
        wom = const.tile([P, K * 32], BF16, tag="wom", name="wom")
        nc.sync.dma_start(_r3(wom, 32), wom_d[:, :, :].transpose([1, 0, 2]))
        wdef = const.tile([P, K * P], BF16, tag="wdef", name="wdef")
        nc.sync.dma_start(_r3(wdef, P), wdef_d[:, :, :].transpose([1, 0, 2]))
        wfus = const.tile([P, 2 * P], BF16, tag="wfus", name="wfus")
        nc.sync.dma_start(_r3(wfus, P), wfus_d[:, :, :].transpose([1, 0, 2]))
        bfus = const.tile([P, 1], F32, tag="bfus", name="bfus")
        nc.sync.dma_start(bfus[:, :], bfus_d[:, :])
        bom = const.tile([32, 1], F32, tag="bom", name="bom")
        nc.sync.dma_start(bom[:, :], bom_d[:, :])
        yb = const.tile([P, K * 32], F32, tag="yb", name="yb")
        nc.sync.dma_start(yb[:, :], yb_d[:, :])
        xb = const.tile([P, K * 32], F32, tag="xb", name="xb")
        nc.sync.dma_start(xb[:, :], xb_d[:, :])
        idn = const.tile([P, P], BF16, tag="idn", name="idn")
        nc.sync.dma_start(idn[:, :], idn_d[:, :])
        idnf = const.tile([P, P], F32, tag="idnf", name="idnf")
        nc.sync.dma_start(idnf[:, :], idnf_d[:, :])

        # ---- long-lived tiles ----
        xp = main.tile([P, NPAD], BF16, tag="xp", name="xp")
        xstd = main.tile([P, NP], BF16, tag="xstd", name="xstd")
        pm = main.tile([P, 32 * 32], F32, tag="pm", name="pm")
        idx32 = main.tile([P, K * 32], mybir.dt.int32, tag="idx32", name="idx32")
        idx16 = main.tile([P, K * 32], I16, tag="idx16", name="idx16")
        # wrapped int16 indices for dma_gather: per (w,k) slice
        # [:, (k*2+w)*128 : +128]; idxw[q, kw*128 + 8j + r] =
        # idx16[16r + q, k*32 + w*16 + j]; replicated to all 128 partitions.
        idxw = main.tile([P, K * 2 * 128], I16, tag="idxw", name="idxw")
        # corner scales, corner-interleaved + pair-duplicated:
        # col = (k*32+b)*8 + corner*2 + {0,1}; corner order matches the
        # gathered patch layout [v00 | v10 | v01 | v11].
        sall = main.tile([P, K * 32 * 8], BF16, tag="sall", name="sall")
        xdef = main.tile([P, NP], BF16, tag="xdef", name="xdef")

        # ================= phase A: load + padded bf16 image ============
        with tc.tile_pool(name="ph_a", bufs=1) as pa:
            x_sb = pa.tile([P, NP], F32, tag="x_sb", name="x_sb")
            nc.sync.dma_start(x_sb[:, :], x_d[:, :])
            nc.vector.memset(xp[:, :], 0.0)
            nc.vector.tensor_copy(
                _r3(xp, HP)[:, 2 : 2 + H, 2 : 2 + H],
                _r3(x_sb, H),
            )

            # ============= phase B: channel-last row-pair image =========
            xcl = pa.tile([P, NBLK * P], BF16, tag="xcl", name="xcl")
            nc.vector.memset(xcl[:, :], 0.0)
            for pb in range(NBLK):
                n = min(P, NPAD - pb * P)
                tp = ps_tr.tile([P, 256], BF16, tag="tp", name="tpb")
                nc.tensor.transpose(tp[:n, :P], xp[:, pb * P : pb * P + n], idn)
                nc.scalar.activation(xcl[:n, pb * P : (pb + 1) * P], tp[:n, :P], AF.Copy)
            x4r = x4_d[:, :].rearrange("(b p) c -> p b c", p=P)  # [128, 37, 512]
            xclr = _r3(xcl, P)  # [128, 37, 128]
            # plane 0: entry (r,c) <- x_cl[r,c]
            nc.sync.dma_start(x4r[:, :, 0:P], xclr)
            # plane 1: entry (r,c) <- x_cl[r+1,c]  (source shifted by 68 pos)
            nc.sync.dma_start(x4r[0:60, :, P:256], xclr[68:128, :, :])
            nc.sync.dma_start(x4r[60:128, 0 : NBLK - 1, P:256], xclr[0:68, 1:NBLK, :])
            # plane 2: entry (r,c) <- x_cl[r,c+1]  (shift by 1 pos)
            nc.sync.dma_start(x4r[0:127, :, 256:384], xclr[1:128, :, :])
            nc.sync.dma_start(x4r[127:128, 0 : NBLK - 1, 256:384], xclr[0:1, 1:NBLK, :])
            # plane 3: entry (r,c) <- x_cl[r+1,c+1]  (shift by 69 pos)
            nc.sync.dma_start(x4r[0:59, :, 384:512], xclr[69:128, :, :])
            nc.sync.dma_start(x4r[59:128, 0 : NBLK - 1, 384:512], xclr[0:69, 1:NBLK, :])

        # ================= phase C: convs ===============================
        def conv_rhs(n, ki, kj):
            base = (8 * n + ki + 1) * HP
            v = xp[:, base : base + 8 * HP]
            return _r3(v, HP)[:, :, kj + 1 : kj + 1 + H]

        ps_conv = top.enter_context(tc.tile_pool(name="ps_conv", bufs=2, space="PSUM"))
        with tc.tile_pool(name="ph_c", bufs=1) as pc:
            om = pc.tile([32, NP], F32, tag="om", name="om")
            # offset/modulator conv first: it gates phases D/E/G.
            for n in range(8):
                ps = ps_conv.tile([P, 512], F32, tag="ps_c", name="ps_c")
                for k in range(K):
                    nc.tensor.matmul(
                        ps[:32, :], wom[:, k * 32 : (k + 1) * 32],
                        conv_rhs(n, k // 3, k % 3),
                        start=(k == 0), stop=(k == K - 1),
                    )
                nc.scalar.activation(
                    om[:, n * 512 : (n + 1) * 512], ps[:32, :], AF.Identity, bias=bom[:, :]
                )

            # ============= phase D: transpose offmod to position-major ==
            for b in range(32):
                tp = ps_tr.tile([P, 256], F32, tag="tp", name="tpd")
                nc.tensor.transpose(
                    tp[:, :32], om[:, b * P : (b + 1) * P], idnf[:32, :32]
                )
                nc.vector.tensor_copy(pm[:, b * 32 : (b + 1) * 32], tp[:, :32])

        # ================= phase E: index & weight math =================
        pmr = pm.rearrange("p (b c) -> p c b", c=32)  # [128, ch32, b32]
        with tc.tile_pool(name="ph_e", bufs=1) as pe:
            def t288(tag, dt=F32):
                return pe.tile([P, K * 32], dt, tag=tag, name=tag)

            py = t288("py"); px = t288("px")
            iy = t288("iy"); ix = t288("ix")
            wy = t288("wy"); wx = t288("wx")
            u = t288("u"); vv = t288("vv")
            a = t288("a"); bw = t288("bw")
            m = t288("m")
            idxf = t288("idxf")
            sg = pe.tile([P, 13 * 32], F32, tag="sg", name="sg")

            v3 = lambda t: _r3(t, 32)  # [128, 9, 32]

            # py = dy + ybase ; px = dx + xbase
            nc.vector.tensor_tensor(v3(py), pmr[:, 0:18:2, :], v3(yb), op=OP.add)
            nc.vector.tensor_tensor(v3(px), pmr[:, 1:19:2, :], v3(xb), op=OP.add)
            for t in (py, px):
                nc.vector.tensor_scalar(
                    t[:, :], t[:, :], 66.4, 0.6, op0=OP.min, op1=OP.max
                )
            # floor via round-to-nearest(v - 0.5)
            nc.vector.tensor_scalar(iy[:, :], py[:, :], 0.5, MAGIC, op0=OP.subtract, op1=OP.add)
            nc.vector.tensor_scalar(iy[:, :], iy[:, :], MAGIC, None, op0=OP.subtract)
            nc.vector.tensor_scalar(ix[:, :], px[:, :], 0.5, MAGIC, op0=OP.subtract, op1=OP.add)
            nc.vector.tensor_scalar(ix[:, :], ix[:, :], MAGIC, None, op0=OP.subtract)
            nc.vector.tensor_tensor(wy[:, :], py[:, :], iy[:, :], op=OP.subtract)
            nc.vector.tensor_tensor(wx[:, :], px[:, :], ix[:, :], op=OP.subtract)
            # gather index = iy*68 + ix
            nc.vector.tensor_scalar(idxf[:, :], iy[:, :], 68.0, None, op0=OP.mult)
            nc.vector.tensor_tensor(idxf[:, :], idxf[:, :], ix[:, :], op=OP.add)
            nc.vector.tensor_copy(idx32[:, :], idxf[:, :])
            # int16 indices, rewrapped via DRAM bounce into the [16, n/16]
            # layout dma_gather expects, then replicated to all partitions.
            nc.vector.tensor_copy(idx16[:, :], idxf[:, :])
            nc.sync.dma_start(idxd_d[:, :], idx16[:, :])
            nc.sync.dma_start(
                idxw[0:16, :].rearrange("q (a r) -> q a r", r=8),
                idxd_d[:, :].rearrange("(r q) a -> q a r", q=16),
            )
            nc.sync.dma_start(idxw[16:32, :], idxw[0:16, :])
            nc.sync.dma_start(idxw[32:64, :], idxw[0:32, :])
            nc.sync.dma_start(idxw[64:128, :], idxw[0:64, :])

            # mask: sigmoid(std_mod) * sigmoid(corner sel; absent taps -> 0.5)
            nc.scalar.activation(_r3(sg, 32), pmr[:, 18:31, :], AF.Sigmoid)
            sgr = _r3(sg, 32)  # [128, 13, 32]
            for ci, k in enumerate((0, 2, 6, 8)):
                nc.vector.tensor_tensor(
                    m[:, k * 32 : (k + 1) * 32], sgr[:, k, :], sgr[:, 9 + ci, :],
                    op=OP.mult,
                )
            for k in (1, 3, 4, 5, 7):
                nc.vector.tensor_scalar(
                    m[:, k * 32 : (k + 1) * 32], sgr[:, k, :], 0.5, None, op0=OP.mult
                )

            # corner scales (mask folded): s_cr = m * wy_part * wx_part,
            # written into sall corner-interleaved + pair-duplicated.
            nc.vector.tensor_scalar(u[:, :], wy[:, :], -1.0, 1.0, op0=OP.mult, op1=OP.add)
            nc.vector.tensor_scalar(vv[:, :], wx[:, :], -1.0, 1.0, op0=OP.mult, op1=OP.add)
            nc.vector.tensor_tensor(a[:, :], m[:, :], u[:, :], op=OP.mult)    # (1-wy)*m
            nc.vector.tensor_tensor(bw[:, :], m[:, :], wy[:, :], op=OP.mult)  # wy*m
            sall8 = sall.rearrange("p (kb e) -> p kb e", e=8)  # [128, 288, 8]
            pair = lambda t: t.rearrange("p (n o) -> p n o", o=1).broadcast_to(
                (P, K * 32, 2)
            )
            # corner order = gathered patch layout [v00 | v10 | v01 | v11]
            nc.vector.tensor_tensor(sall8[:, :, 0:2], pair(a), pair(vv), op=OP.mult)
            nc.vector.tensor_tensor(sall8[:, :, 2:4], pair(bw), pair(vv), op=OP.mult)
            nc.vector.tensor_tensor(sall8[:, :, 4:6], pair(a), pair(wx), op=OP.mult)
            nc.vector.tensor_tensor(sall8[:, :, 6:8], pair(bw), pair(wx), op=OP.mult)

        # ============== phase C2: std conv (overlaps gather ramp) =======
        for n in range(8):
            ps = ps_conv.tile([P, 512], F32, tag="ps_c", name="ps_c")
            for k in range(K):
                nc.tensor.matmul(
                    ps[:, :], wstd[:, k * P : (k + 1) * P],
                    conv_rhs(n, k // 3, k % 3),
                    start=(k == 0), stop=(k == K - 1),
                )
            nc.scalar.activation(xstd[:, n * 512 : (n + 1) * 512], ps[:, :], AF.Copy)

        # ================= phase G: gather + combine + def conv =========
        import os
        gather_mode = os.environ.get("KB_GATHER", "dmagather")
        new_combine = os.environ.get("KB_NEW_COMBINE", "1") == "1"
        with tc.tile_pool(name="gpool", bufs=3) as gpool, \
             tc.tile_pool(name="qpool", bufs=1) as qpool, \
             tc.tile_pool(name="spool", bufs=2) as spool, \
             tc.tile_pool(name="ps_def", bufs=1, space="PSUM") as ps_def:
            for w in range(NW):
                psd = ps_def.tile([P, WBLK * P], F32, tag="psd", name="psd")
                for k in range(K):
                    c0 = k * 32 + w * WBLK
                    g = gpool.tile([P, WBLK, 512], BF16, tag="g", name="g")
                    if gather_mode == "dmagather":
                        kw = k * 2 + w
                        nc.gpsimd.dma_gather(
                            out_ap=g[:, :, :],
                            in_ap=x4_d[:, :],
                            idxs_ap=idxw[:, kw * 128 : (kw + 1) * 128],
                            num_idxs=WBLK * 128,
                            num_idxs_reg=WBLK * 128,
                            elem_size=512,
                        )
                    else:
                        for bb in range(WBLK):
                            nc.gpsimd.indirect_dma_start(
                                out=g[:, bb, :],
                                out_offset=None,
                                in_=x4_d[:, :],
                                in_offset=bass.IndirectOffsetOnAxis(
                                    ap=idx32[:, c0 + bb : c0 + bb + 1], axis=0
                                ),
                            )
                    # combine: q = g * scales (2x), then 4:1 tree add (2x)
                    q = qpool.tile([P, WBLK * 512], BF16, tag="q", name="q")
                    t = qpool.tile([P, WBLK * 256], BF16, tag="t", name="t")
                    samp = spool.tile([P, WBLK * P], BF16, tag="samp", name="samp")
                    if new_combine:
                        gv = g[:, :, :].rearrange("p b (c x e) -> p (b c) x e", c=4, e=2)
                        sv = (
                            sall[:, c0 * 8 : (c0 + WBLK) * 8]
                            .rearrange("p (bc o e) -> p bc o e", o=1, e=2)
                            .broadcast_to((P, WBLK * 4, 64, 2))
                        )
                        qv = q.rearrange("p (bc x e) -> p bc x e", bc=WBLK * 4, e=2)
                        nc.vector.tensor_tensor(qv, gv, sv, op=OP.mult)
                        qh = q.rearrange("p (b h) -> p b h", h=512)
                        th = t.rearrange("p (b h) -> p b h", h=256)
                        nc.vector.tensor_tensor(th, qh[:, :, 0:256], qh[:, :, 256:512], op=OP.add)
                        sh = samp.rearrange("p (b h) -> p b h", h=128)
                        nc.vector.tensor_tensor(sh, th[:, :, 0:128], th[:, :, 128:256], op=OP.add)
                    else:
                        # baseline-style combine: per-corner bcast mult + add,
                        # scales read from sall (stride-8 pair views).
                        sampv = samp.rearrange("p (b o) -> p b o", o=P)
                        qb = q.rearrange("p (b o) -> p b o", o=512)
                        s8v = sall.rearrange("p (kb e) -> p kb e", e=8)
                        for j, cr in enumerate(range(4)):
                            src = g[:, :, cr * P : (cr + 1) * P]
                            sbc = (
                                s8v[:, c0 : c0 + WBLK, 2 * cr : 2 * cr + 1]
                                .broadcast_to((P, WBLK, P))
                            )
                            if j == 0:
                                nc.vector.tensor_tensor(sampv, src, sbc, op=OP.mult)
                            else:
                                nc.vector.tensor_tensor(qb[:, :, 0:P], src, sbc, op=OP.mult)
                                nc.vector.tensor_tensor(sampv, sampv, qb[:, :, 0:P], op=OP.add)

                    rhsT = spool.tile([P, WBLK * P], BF16, tag="rhsT", name="rhsT")
                    for bb in range(WBLK):
                        tp = ps_tr.tile([P, 256], BF16, tag="tp", name="tpg")
                        nc.tensor.transpose(
                            tp[:, :P], samp[:, bb * P : (bb + 1) * P], idn
                        )
                        nc.scalar.activation(
                            rhsT[:, bb * P : (bb + 1) * P], tp[:, :P], AF.Copy
                        )
                    for bb in range(WBLK):
                        # start marks the whole 2KB PSUM bank (4 blocks)
                        # pending-zero, so only the first block of each bank
                        # may set it.
                        nc.tensor.matmul(
                            psd[:, bb * P : (bb + 1) * P],
                            wdef[:, k * P : (k + 1) * P],
                            rhsT[:, bb * P : (bb + 1) * P],
                            start=(k == 0 and bb % 4 == 0),
                            stop=(k == K - 1 and bb % 4 == 3),
                            skip_group_check=True,
                        )
                nc.scalar.activation(
                    xdef[:, w * WBLK * P : (w + 1) * WBLK * P], psd[:, :], AF.Copy
                )

        # ================= phase H: fused 1x1 conv ======================
        with tc.tile_pool(name="ps_fus", bufs=2, space="PSUM") as ps_fus, \
             tc.tile_pool(name="ph_h", bufs=2) as ph:
            for n in range(8):
                ps = ps_fus.tile([P, 512], F32, tag="ps_h", name="ps_h")
                nc.tensor.matmul(
                    ps[:, :], wfus[:, 0:P], xstd[:, n * 512 : (n + 1) * 512],
                    start=True, stop=False,
                )
                nc.tensor.matmul(
                    ps[:, :], wfus[:, P : 2 * P], xdef[:, n * 512 : (n + 1) * 512],
                    start=False, stop=True,
                )
                stage = ph.tile([P, 512], F32, tag="stage", name="stage")
                nc.scalar.activation(stage[:, :], ps[:, :], AF.Identity, bias=bfus[:, :])
                nc.sync.dma_start(out_d[:, n * 512 : (n + 1) * 512], stage[:, :])

    return nc


def _consts(W_std, b_std, W_off, b_off, W_mod, b_mod, W_def, b_def, W_fus, b_fus):
    """Host-side constant prep (shared across cores)."""
    f = np.float32
    wstd = np.transpose(W_std, (2, 3, 1, 0)).reshape(K, P, P)  # [k, c, o]
    wom_full = np.concatenate([W_off, W_mod], axis=0)  # [31, 128, 3, 3]
    wom = np.zeros((K, P, 32), f)
    wom[:, :, :31] = np.transpose(wom_full, (2, 3, 1, 0)).reshape(K, P, 31)
    wdef = np.transpose(W_def, (2, 3, 1, 0)).reshape(K, P, P)
    wf = W_fus[:, :, 0, 0]  # [128, 256]
    wfus = np.stack([wf[:, :P].T, wf[:, P:].T], axis=0)  # [2, c, o]
    bfus = (b_fus + wf[:, :P] @ b_std + wf[:, P:] @ b_def).reshape(P, 1)
    bom = np.zeros((32, 1), f)
    bom[:18, 0] = b_off
    bom[18:31, 0] = b_mod
    # ybase/xbase in [p, k*32+b] layout: j = b*128 + p
    pp, kk, bb2 = np.meshgrid(np.arange(P), np.arange(K), np.arange(32), indexing="ij")
    j = bb2 * 128 + pp
    yb = ((j >> 6) + (kk // 3) + 1).astype(f).reshape(P, K * 32)
    xb = ((j & 63) + (kk % 3) + 1).astype(f).reshape(P, K * 32)
    return dict(
        wstd=wstd.astype(NPBF), wom=wom.astype(NPBF), wdef=wdef.astype(NPBF),
        wfus=wfus.astype(NPBF), bfus=bfus.astype(f), bom=bom.astype(f),
        yb=yb, xb=xb,
        idn=np.eye(P, dtype=NPBF), idnf=np.eye(P, dtype=f),
    )


_NC_CACHE = {}


def _get_nc():
    if "nc" not in _NC_CACHE:
        nc = build_nc()
        nc.finalize()
        _NC_CACHE["nc"] = nc
    return _NC_CACHE["nc"]


def kernel(x, W_std, b_std, W_off, b_off, W_corner, b_corner, W_mod, b_mod,
           W_def, b_def, W_fus, b_fus, **kw):
    consts = _consts(
        np.asarray(W_std, np.float32), np.asarray(b_std, np.float32),
        np.asarray(W_off, np.float32), np.asarray(b_off, np.float32),
        np.asarray(W_mod, np.float32), np.asarray(b_mod, np.float32),
        np.asarray(W_def, np.float32), np.asarray(b_def, np.float32),
        np.asarray(W_fus, np.float32), np.asarray(b_fus, np.float32),
    )
    x = np.asarray(x, np.float32)
    B = x.shape[0]
    assert B == N_CORES, x.shape
    in_maps = []
    for b in range(B):
        im = dict(consts)
        im["x"] = np.ascontiguousarray(x[b].reshape(P, NP))
        in_maps.append(im)
    nc = _get_nc()
    res = run_bass_kernel_spmd(nc, in_maps, core_ids=list(range(N_CORES)))
    out = np.stack([r["out"].reshape(P, H, H) for r in res.results], axis=0)
    return out.astype(np.float32)


if __name__ == "__main__":
    nc = build_nc()
    print("built ok:", len(nc.m.functions[0].instructions)
          if hasattr(nc.m.functions[0], "instructions") else "?")


# revision 23
# speedup vs baseline: 1.2086x; 1.2086x over previous
"""Trainium2 Bass kernel for nn_DABConv (deformable attention-ish conv).

Data-parallel over batch: 8 samples -> 8 NeuronCores, one sample per core.

Per-core pipeline (sample = x[C=128, H=64, W=64] fp32):
  A. load x, build zero-padded bf16 image xp [128, 68*68] (pad=2)
  B. build channel-last "row-pair" image x2cl in DRAM:
     x2cl[(r,c)] = [x_cl[r,c](128ch), x_cl[r+1,c](128ch)] bf16, so one
     1KB gathered element = a full 2x2 bilinear patch.
  C. convs (std 128ch, offset+modulator 31ch) as 9 accumulating shifted
     matmuls each, bf16.  Offset/mod conv is emitted FIRST (it gates the
     gather loop); the std conv is emitted after phase E so the PE chews
     it while gathers run.
  D. PE-transpose offset/mod output to position-major layout pm.
  E. index + bilinear-weight math on DVE (position-major, fp32),
     fold mask*corner-gate into the 4 corner scales; corner scales are
     written corner-interleaved and pair-duplicated (sall) so the
     phase-G combine multiplies run with packed last-dim APs (DVE 2x).
  G. per (wave, tap): ONE batched indirect DMA gathers 16 blocks x 128
     patches (2048 x 1KB) -> 3 DVE ops (scaled corner mult + 2 tree
     adds) -> PE transpose to channel-major -> accumulating def-conv
     matmul into PSUM.
  H. fused 1x1 conv over [x_std ; x_def], biases folded host-side.
"""

import numpy as np
import ml_dtypes
from contextlib import ExitStack

import concourse.bass as bass
import concourse.bacc as bacc
import concourse.mybir as mybir
from concourse.tile import TileContext
from concourse.bass_utils import run_bass_kernel_spmd

AF = mybir.ActivationFunctionType
OP = mybir.AluOpType
F32 = mybir.dt.float32
BF16 = mybir.dt.bfloat16
I16 = mybir.dt.int16
NPBF = ml_dtypes.bfloat16

P = 128
H = 64
HP = 68          # padded image side (pad=2 each side)
NP = H * H       # 4096 output positions
NPAD = HP * HP   # 4624 padded positions
NBLK = 37        # ceil(4624/128)
K = 9
NW = 2           # waves over position blocks (PSUM capacity)
WBLK = 16        # 128-position blocks per wave
MAGIC = 12582912.0  # 2**23 + 2**22: float32 round-to-int trick
N_CORES = 8


def _r3(ap, inner):
    """[p, (a b)] -> [p, a, b] with b=inner."""
    return ap.rearrange("p (a b) -> p a b", b=inner)


def build_nc():
    nc = bacc.Bacc("TRN2", target_bir_lowering=False, debug=False)

    x_d = nc.dram_tensor("x", [P, NP], F32, kind="ExternalInput")
    wstd_d = nc.dram_tensor("wstd", [K, P, P], BF16, kind="ExternalInput")
    wom_d = nc.dram_tensor("wom", [K, P, 32], BF16, kind="ExternalInput")
    wdef_d = nc.dram_tensor("wdef", [K, P, P], BF16, kind="ExternalInput")
    wfus_d = nc.dram_tensor("wfus", [2, P, P], BF16, kind="ExternalInput")
    bfus_d = nc.dram_tensor("bfus", [P, 1], F32, kind="ExternalInput")
    bom_d = nc.dram_tensor("bom", [32, 1], F32, kind="ExternalInput")
    yb_d = nc.dram_tensor("yb", [P, K * 32], F32, kind="ExternalInput")
    xb_d = nc.dram_tensor("xb", [P, K * 32], F32, kind="ExternalInput")
    idn_d = nc.dram_tensor("idn", [P, P], BF16, kind="ExternalInput")
    idnf_d = nc.dram_tensor("idnf", [P, P], F32, kind="ExternalInput")
    out_d = nc.dram_tensor("out", [P, NP], F32, kind="ExternalOutput")
    # internal scratch: channel-last row-pair image; row j = padded pos
    # (r,c) holds [x(r,c) | x(r+1,c)] x 128ch bf16; a 512-elem gather at
    # row j spans rows j, j+1 = the full 2x2 bilinear patch.
    x2_d = nc.dram_tensor("x2cl", [NBLK * 128, 256], BF16)

    with TileContext(nc) as tc, ExitStack() as top:
        const = top.enter_context(tc.tile_pool(name="const", bufs=1))
        main = top.enter_context(tc.tile_pool(name="main", bufs=1))
        ps_tr = top.enter_context(tc.tile_pool(name="ps_tr", bufs=2, space="PSUM"))

        # ---- const loads ----
        wstd = const.tile([P, K * P], BF16, tag="wstd", name="wstd")
        nc.sync.dma_start(_r3(wstd, P), wstd_d[:, :, :].transpose([1, 0, 2]))
        wom = const.tile([P, K * 32], BF16, tag="wom", name="wom")
        nc.sync.dma_start(_r3(wom, 32), wom_d[:, :, :].transpose([1, 0, 2]))
        wdef = const.tile([P, K * P], BF16, tag="wdef", name="wdef")
        nc.sync.dma_start(_r3(wdef, P), wdef_d[:, :, :].transpose([1, 0, 2]))
        wfus = const.tile([P, 2 * P], BF16, tag="wfus", name="wfus")
        nc.sync.dma_start(_r3(wfus, P), wfus_d[:, :, :].transpose([1, 0, 2]))
        bfus = const.tile([P, 1], F32, tag="bfus", name="bfus")
        nc.sync.dma_start(bfus[:, :], bfus_d[:, :])
        bom = const.tile([32, 1], F32, tag="bom", name="bom")
        nc.sync.dma_start(bom[:, :], bom_d[:, :])
        yb = const.tile([P, K * 32], F32, tag="yb", name="yb")
        nc.sync.dma_start(yb[:, :], yb_d[:, :])
        xb = const.tile([P, K * 32], F32, tag="xb", name="xb")
        nc.sync.dma_start(xb[:, :], xb_d[:, :])
        idn = const.tile([P, P], BF16, tag="idn", name="idn")
        nc.sync.dma_start(idn[:, :], idn_d[:, :])
        idnf = const.tile([P, P], F32, tag="idnf", name="idnf")
        nc.sync.dma_start(idnf[:, :], idnf_d[:, :])

        # ---- long-lived tiles ----
        xp = main.tile([P, NPAD], BF16, tag="xp", name="xp")
        xstd = main.tile([P, NP], BF16, tag="xstd", name="xstd")
        pm = main.tile([P, 32 * 32], F32, tag="pm", name="pm")
        idx32 = main.tile([P, K * 32], mybir.dt.int32, tag="idx32", name="idx32")
        # corner scales, corner-interleaved + pair-duplicated:
        # col = (k*32+b)*8 + corner*2 + {0,1}; corner order matches the
        # gathered patch layout [v00 | v10 | v01 | v11].
        sall = main.tile([P, K * 32 * 8], BF16, tag="sall", name="sall")
        xdef = main.tile([P, NP], BF16, tag="xdef", name="xdef")

        # ================= phase A: load + padded bf16 image ============
        with tc.tile_pool(name="ph_a", bufs=1) as pa:
            x_sb = pa.tile([P, NP], F32, tag="x_sb", name="x_sb")
            nc.sync.dma_start(x_sb[:, :], x_d[:, :])
            nc.vector.memset(xp[:, :], 0.0)
            nc.vector.tensor_copy(
                _r3(xp, HP)[:, 2 : 2 + H, 2 : 2 + H],
                _r3(x_sb, H),
            )

            # ============= phase B: channel-last row-pair image =========
            xcl = pa.tile([P, NBLK * P], BF16, tag="xcl", name="xcl")
            nc.vector.memset(xcl[:, :], 0.0)
            for pb in range(NBLK):
                n = min(P, NPAD - pb * P)
                tp = ps_tr.tile([P, 256], BF16, tag="tp", name="tpb")
                nc.tensor.transpose(tp[:n, :P], xp[:, pb * P : pb * P + n], idn)
                nc.scalar.activation(xcl[:n, pb * P : (pb + 1) * P], tp[:n, :P], AF.Copy)
            x2r = x2_d[:, :].rearrange("(b p) c -> p b c", p=P)  # [128, 37, 256]
            xclr = _r3(xcl, P)  # [128, 37, 128]
            # plane 0: entry (r,c) <- x_cl[r,c]
            nc.sync.dma_start(x2r[:, :, 0:P], xclr)
            # plane 1: entry (r,c) <- x_cl[r+1,c]  (source shifted by 68 pos)
            nc.sync.dma_start(x2r[0:60, :, P:256], xclr[68:128, :, :])
            nc.sync.dma_start(x2r[60:128, 0 : NBLK - 1, P:256], xclr[0:68, 1:NBLK, :])

        # ================= phase C: convs ===============================
        def conv_rhs(n, ki, kj):
            base = (8 * n + ki + 1) * HP
            v = xp[:, base : base + 8 * HP]
            return _r3(v, HP)[:, :, kj + 1 : kj + 1 + H]

        ps_conv = top.enter_context(tc.tile_pool(name="ps_conv", bufs=2, space="PSUM"))
        with tc.tile_pool(name="ph_c", bufs=1) as pc:
            om = pc.tile([32, NP], F32, tag="om", name="om")
            # offset/modulator conv first: it gates phases D/E/G.
            for n in range(8):
                ps = ps_conv.tile([P, 512], F32, tag="ps_c", name="ps_c")
                for k in range(K):
                    nc.tensor.matmul(
                        ps[:32, :], wom[:, k * 32 : (k + 1) * 32],
                        conv_rhs(n, k // 3, k % 3),
                        start=(k == 0), stop=(k == K - 1),
                    )
                nc.scalar.activation(
                    om[:, n * 512 : (n + 1) * 512], ps[:32, :], AF.Identity, bias=bom[:, :]
                )

            # ============= phase D: transpose offmod to position-major ==
            for b in range(32):
                tp = ps_tr.tile([P, 256], F32, tag="tp", name="tpd")
                nc.tensor.transpose(
                    tp[:, :32], om[:, b * P : (b + 1) * P], idnf[:32, :32]
                )
                nc.vector.tensor_copy(pm[:, b * 32 : (b + 1) * 32], tp[:, :32])

        # ================= phase E: index & weight math =================
        pmr = pm.rearrange("p (b c) -> p c b", c=32)  # [128, ch32, b32]
        with tc.tile_pool(name="ph_e", bufs=1) as pe:
            def t288(tag, dt=F32):
                return pe.tile([P, K * 32], dt, tag=tag, name=tag)

            py = t288("py"); px = t288("px")
            iy = t288("iy"); ix = t288("ix")
            wy = t288("wy"); wx = t288("wx")
            u = t288("u"); vv = t288("vv")
            a = t288("a"); bw = t288("bw")
            m = t288("m")
            idxf = t288("idxf")
            sg = pe.tile([P, 13 * 32], F32, tag="sg", name="sg")

            v3 = lambda t: _r3(t, 32)  # [128, 9, 32]

            # py = dy + ybase ; px = dx + xbase
            nc.vector.tensor_tensor(v3(py), pmr[:, 0:18:2, :], v3(yb), op=OP.add)
            nc.vector.tensor_tensor(v3(px), pmr[:, 1:19:2, :], v3(xb), op=OP.add)
            for t in (py, px):
                nc.vector.tensor_scalar(
                    t[:, :], t[:, :], 66.4, 0.6, op0=OP.min, op1=OP.max
                )
            # floor via round-to-nearest(v - 0.5)
            nc.vector.tensor_scalar(iy[:, :], py[:, :], 0.5, MAGIC, op0=OP.subtract, op1=OP.add)
            nc.vector.tensor_scalar(iy[:, :], iy[:, :], MAGIC, None, op0=OP.subtract)
            nc.vector.tensor_scalar(ix[:, :], px[:, :], 0.5, MAGIC, op0=OP.subtract, op1=OP.add)
            nc.vector.tensor_scalar(ix[:, :], ix[:, :], MAGIC, None, op0=OP.subtract)
            nc.vector.tensor_tensor(wy[:, :], py[:, :], iy[:, :], op=OP.subtract)
            nc.vector.tensor_tensor(wx[:, :], px[:, :], ix[:, :], op=OP.subtract)
            # gather index = iy*68 + ix
            nc.vector.tensor_scalar(idxf[:, :], iy[:, :], 68.0, None, op0=OP.mult)
            nc.vector.tensor_tensor(idxf[:, :], idxf[:, :], ix[:, :], op=OP.add)
            nc.vector.tensor_copy(idx32[:, :], idxf[:, :])

            # mask: sigmoid(std_mod) * sigmoid(corner sel; absent taps -> 0.5)
            nc.scalar.activation(_r3(sg, 32), pmr[:, 18:31, :], AF.Sigmoid)
            sgr = _r3(sg, 32)  # [128, 13, 32]
            for ci, k in enumerate((0, 2, 6, 8)):
                nc.vector.tensor_tensor(
                    m[:, k * 32 : (k + 1) * 32], sgr[:, k, :], sgr[:, 9 + ci, :],
                    op=OP.mult,
                )
            for k in (1, 3, 4, 5, 7):
                nc.vector.tensor_scalar(
                    m[:, k * 32 : (k + 1) * 32], sgr[:, k, :], 0.5, None, op0=OP.mult
                )

            # corner scales (mask folded): s_cr = m * wy_part * wx_part,
            # written into sall corner-interleaved + pair-duplicated.
            nc.vector.tensor_scalar(u[:, :], wy[:, :], -1.0, 1.0, op0=OP.mult, op1=OP.add)
            nc.vector.tensor_scalar(vv[:, :], wx[:, :], -1.0, 1.0, op0=OP.mult, op1=OP.add)
            nc.vector.tensor_tensor(a[:, :], m[:, :], u[:, :], op=OP.mult)    # (1-wy)*m
            nc.vector.tensor_tensor(bw[:, :], m[:, :], wy[:, :], op=OP.mult)  # wy*m
            sall8 = sall.rearrange("p (kb e) -> p kb e", e=8)  # [128, 288, 8]
            pair = lambda t: t.rearrange("p (n o) -> p n o", o=1).broadcast_to(
                (P, K * 32, 2)
            )
            # corner order = gathered patch layout [v00 | v10 | v01 | v11]
            nc.vector.tensor_tensor(sall8[:, :, 0:2], pair(a), pair(vv), op=OP.mult)
            nc.vector.tensor_tensor(sall8[:, :, 2:4], pair(bw), pair(vv), op=OP.mult)
            nc.vector.tensor_tensor(sall8[:, :, 4:6], pair(a), pair(wx), op=OP.mult)
            nc.vector.tensor_tensor(sall8[:, :, 6:8], pair(bw), pair(wx), op=OP.mult)

        # ============== phase C2: std conv (overlaps gather ramp) =======
        for n in range(8):
            ps = ps_conv.tile([P, 512], F32, tag="ps_c", name="ps_c")
            for k in range(K):
                nc.tensor.matmul(
                    ps[:, :], wstd[:, k * P : (k + 1) * P],
                    conv_rhs(n, k // 3, k % 3),
                    start=(k == 0), stop=(k == K - 1),
                )
            nc.scalar.activation(xstd[:, n * 512 : (n + 1) * 512], ps[:, :], AF.Copy)

        # ================= phase G: gather + combine + def conv =========
        with tc.tile_pool(name="gpool", bufs=3) as gpool, \
             tc.tile_pool(name="qpool", bufs=1) as qpool, \
             tc.tile_pool(name="spool", bufs=2) as spool, \
             tc.tile_pool(name="ps_def", bufs=1, space="PSUM") as ps_def:
            for w in range(NW):
                psd = ps_def.tile([P, WBLK * P], F32, tag="psd", name="psd")
                for k in range(K):
                    c0 = k * 32 + w * WBLK
                    g = gpool.tile([P, WBLK, 512], BF16, tag="g", name="g")
                    for bb in range(WBLK):
                        nc.gpsimd.indirect_dma_start(
                            out=g[:, bb, :],
                            out_offset=None,
                            in_=x2_d[:, :],
                            in_offset=bass.IndirectOffsetOnAxis(
                                ap=idx32[:, c0 + bb : c0 + bb + 1], axis=0
                            ),
                        )
                    # combine: q = g * scales (2x), then 4:1 tree add (2x)
                    q = qpool.tile([P, WBLK * 512], BF16, tag="q", name="q")
                    t = qpool.tile([P, WBLK * 256], BF16, tag="t", name="t")
                    samp = spool.tile([P, WBLK * P], BF16, tag="samp", name="samp")
                    gv = g[:, :, :].rearrange(
                        "p b (c x e) -> p (b c) x e", c=4, e=2
                    )
                    sv = (
                        sall[:, c0 * 8 : (c0 + WBLK) * 8]
                        .rearrange("p (bc o e) -> p bc o e", o=1, e=2)
                        .broadcast_to((P, WBLK * 4, 64, 2))
                    )
                    qv = q.rearrange("p (bc x e) -> p bc x e", bc=WBLK * 4, e=2)
                    nc.vector.tensor_tensor(qv, gv, sv, op=OP.mult)
                    qh = q.rearrange("p (b h) -> p b h", h=512)
                    th = t.rearrange("p (b h) -> p b h", h=256)
                    nc.vector.tensor_tensor(th, qh[:, :, 0:256], qh[:, :, 256:512], op=OP.add)
                    sh = samp.rearrange("p (b h) -> p b h", h=128)
                    nc.vector.tensor_tensor(sh, th[:, :, 0:128], th[:, :, 128:256], op=OP.add)

                    rhsT = spool.tile([P, WBLK * P], BF16, tag="rhsT", name="rhsT")
                    for bb in range(WBLK):
                        tp = ps_tr.tile([P, 256], BF16, tag="tp", name="tpg")
                        nc.tensor.transpose(
                            tp[:, :P], samp[:, bb * P : (bb + 1) * P], idn
                        )
                        nc.scalar.activation(
                            rhsT[:, bb * P : (bb + 1) * P], tp[:, :P], AF.Copy
                        )
                    for bb in range(WBLK):
                        # start marks the whole 2KB PSUM bank (4 blocks)
                        # pending-zero, so only the first block of each bank
                        # may set it.
                        nc.tensor.matmul(
                            psd[:, bb * P : (bb + 1) * P],
                            wdef[:, k * P : (k + 1) * P],
                            rhsT[:, bb * P : (bb + 1) * P],
                            start=(k == 0 and bb % 4 == 0),
                            stop=(k == K - 1 and bb % 4 == 3),
                            skip_group_check=True,
                        )
                nc.scalar.activation(
                    xdef[:, w * WBLK * P : (w + 1) * WBLK * P], psd[:, :], AF.Copy
                )

        # ================= phase H: fused 1x1 conv ======================
        with tc.tile_pool(name="ps_fus", bufs=2, space="PSUM") as ps_fus, \
             tc.tile_pool(name="ph_h", bufs=2) as ph:
            for n in range(8):
                ps = ps_fus.tile([P, 512], F32, tag="ps_h", name="ps_h")
                nc.tensor.matmul(
                    ps[:, :], wfus[:, 0:P], xstd[:, n * 512 : (n + 1) * 512],
                    start=True, stop=False,
                )
                nc.tensor.matmul(
                    ps[:, :], wfus[:, P : 2 * P], xdef[:, n * 512 : (n + 1) * 512],
                    start=False, stop=True,
                )
                stage = ph.tile([P, 512], F32, tag="stage", name="stage")
                nc.scalar.activation(stage[:, :], ps[:, :], AF.Identity, bias=bfus[:, :])
                nc.sync.dma_start(out_d[:, n * 512 : (n + 1) * 512], stage[:, :])

    return nc


def _consts(W_std, b_std, W_off, b_off, W_mod, b_mod, W_def, b_def, W_fus, b_fus):
    """Host-side constant prep (shared across cores)."""
    f = np.float32
    wstd = np.transpose(W_std, (2, 3, 1, 0)).reshape(K, P, P)  # [k, c, o]
    wom_full = np.concatenate([W_off, W_mod], axis=0)  # [31, 128, 3, 3]
    wom = np.zeros((K, P, 32), f)
    wom[:, :, :31] = np.transpose(wom_full, (2, 3, 1, 0)).reshape(K, P, 31)
    wdef = np.transpose(W_def, (2, 3, 1, 0)).reshape(K, P, P)
    wf = W_fus[:, :, 0, 0]  # [128, 256]
    wfus = np.stack([wf[:, :P].T, wf[:, P:].T], axis=0)  # [2, c, o]
    bfus = (b_fus + wf[:, :P] @ b_std + wf[:, P:] @ b_def).reshape(P, 1)
    bom = np.zeros((32, 1), f)
    bom[:18, 0] = b_off
    bom[18:31, 0] = b_mod
    # ybase/xbase in [p, k*32+b] layout: j = b*128 + p
    pp, kk, bb2 = np.meshgrid(np.arange(P), np.arange(K), np.arange(32), indexing="ij")
    j = bb2 * 128 + pp
    yb = ((j >> 6) + (kk // 3) + 1).astype(f).reshape(P, K * 32)
    xb = ((j & 63) + (kk % 3) + 1).astype(f).reshape(P, K * 32)
    return dict(
        wstd=wstd.astype(NPBF), wom=wom.astype(NPBF), wdef=wdef.astype(NPBF),
        wfus=wfus.astype(NPBF), bfus=bfus.astype(f), bom=bom.astype(f),
        yb=yb, xb=xb,
        idn=np.eye(P, dtype=NPBF), idnf=np.eye(P, dtype=f),
    )


_NC_CACHE = {}


def _get_nc():
    if "nc" not in _NC_CACHE:
        nc = build_nc()
        nc.finalize()
        _NC_CACHE["nc"] = nc
    return _NC_CACHE["nc"]


def kernel(x, W_std, b_std, W_off, b_off, W_corner, b_corner, W_mod, b_mod,
           W_def, b_def, W_fus, b_fus, **kw):
    consts = _consts(
        np.asarray(W_std, np.float32), np.asarray(b_std, np.float32),
        np.asarray(W_off, np.float32), np.asarray(b_off, np.float32),
        np.asarray(W_mod, np.float32), np.asarray(b_mod, np.float32),
        np.asarray(W_def, np.float32), np.asarray(b_def, np.float32),
        np.asarray(W_fus, np.float32), np.asarray(b_fus, np.float32),
    )
    x = np.asarray(x, np.float32)
    B = x.shape[0]
    assert B == N_CORES, x.shape
    in_maps = []
    for b in range(B):
        im = dict(consts)
        im["x"] = np.ascontiguousarray(x[b].reshape(P, NP))
        in_maps.append(im)
    nc = _get_nc()
    res = run_bass_kernel_spmd(nc, in_maps, core_ids=list(range(N_CORES)))
    out = np.stack([r["out"].reshape(P, H, H) for r in res.results], axis=0)
    return out.astype(np.float32)


if __name__ == "__main__":
    nc = build_nc()
    print("built ok:", len(nc.m.functions[0].instructions)
          if hasattr(nc.m.functions[0], "instructions") else "?")
